# revision 1
# baseline (speedup 1.0000x reference)
"""Multi-head attention block (B=2, S=2048, D=1024, H=16) on 8 trn2 cores.

Sharding: core c = (batch b = c//4, head-group g = c%4); each core computes
4 heads of one batch (Megatron column-shard of wq/wk/wv, row-shard of wo,
combined with data-parallel over batch). Host sums the 4 partial outputs
per batch and adds the (folded) bias.

Device-side layout strategy (all matmul contractions live on the partition
axis, no on-chip transposes):
  - activations are fed pre-transposed: qT/kT/vT [D, S]
  - Q,K are projected directly transposed: QT/KT [256, S] (psum = wT.T @ qT)
  - V is projected in natural layout [S, 256] with a ones column appended
    per head (so P @ [V|1] yields both O and the softmax row-sums l)
  - scores are computed transposed: ST[j, i] = KT_h.T @ QT_h, so softmax'
    exp runs on ACT and the PV matmul consumes P without any transpose
  - softmax skips max-subtraction (scores here are O(1); exp is safe in f32)
  - normalization: r = 1/l (DVE), broadcast across partitions on GPSIMD,
    one DVE multiply
  - output projection computed transposed (partial^T [D, S]); host transposes
Matmuls run in float32r (full-rate fp32 path: 8-bit exp / 11-bit mantissa,
~1e-4 element precision). Walrus requires every producer of an f32r matmul
operand to itself write f32r, so all matmul-feeding tiles are allocated
float32r; host pre-rounds the DMA-fed arrays to the f32r grid.
"""

import numpy as np

import concourse.bass as bass
import concourse.mybir as mybir
import concourse.tile as tile
from concourse import bacc
from concourse.bass_utils import run_bass_kernel_spmd

B, S, D, H = 2, 2048, 1024, 16
DK = D // H                  # 64
NCORES = 8
GROUPS = NCORES // B         # 4 head-groups
HPC = H // GROUPS            # 4 heads per core
OL = HPC * DK                # 256 local features
SB = 512                     # query-block (i) width
JB = 128                     # key-block (j) width
NSB = S // SB                # 4
NJB = S // JB                # 16
VS = DK + 1                  # V columns per head incl. ones column (65)

F32 = mybir.dt.float32
F32R = mybir.dt.float32r

NEG = -1e9

LAST_RUN = None  # stash of BassKernelResults for test harness inspection


def _round_f32r(a):
    """Round an f32 array to the f32r grid (top-20-bit float, round to
    nearest) so the raw DMA'd bits are well-rounded f32r."""
    a = np.ascontiguousarray(a, np.float32)
    u = a.view(np.uint32)
    u = (u + 0x7FF + ((u >> 12) & 1)) & np.uint32(0xFFFFF000)
    return u.view(np.float32)


def _classify_mask(mask2):
    """Per (ib, jb) block schedule derived from the boolean mask [S, S]
    (mask2[i, j] True = visible).

    Returns (jlists, bias_tiles):
      jlists[ib] = list of (jb, sub_ops) for j-blocks with any visible entry,
        where sub_ops = list over the 4 i-subblocks (128 wide) of
        ('v', None) visible / ('m', None) fully masked / ('x', bias_idx).
      bias_tiles: [n, JB, 128] f32 additive bias (transposed: [j, i]).
    """
    jlists = []
    bias_tiles = []
    assert mask2.any(axis=1).all(), "mask has a fully-masked query row"
    for ib in range(NSB):
        jl = []
        for jb in range(NJB):
            sub = mask2[ib * SB:(ib + 1) * SB, jb * JB:(jb + 1) * JB]
            if not sub.any():
                continue
            sub_ops = []
            for k in range(SB // 128):
                s2 = sub[k * 128:(k + 1) * 128, :]
                if s2.all():
                    sub_ops.append(("v", None))
                elif not s2.any():
                    sub_ops.append(("m", None))
                else:
                    bias_tiles.append(
                        np.where(s2, np.float32(1), np.float32(0)).T
                    )
                    sub_ops.append(("x", len(bias_tiles) - 1))
            jl.append((jb, sub_ops))
        jlists.append(jl)
    return jlists, bias_tiles


def _exp_runs(sub_ops):
    """Contiguous runs of non-masked i-subblocks: list of (k0, k1)."""
    runs = []
    start = None
    for k, (st, _) in enumerate(sub_ops):
        if st == "m":
            if start is not None:
                runs.append((start, k))
                start = None
        elif start is None:
            start = k
    if start is not None:
        runs.append((start, len(sub_ops)))
    return runs


def _build(jlists, nbias):
    nc = bacc.Bacc()

    qT = nc.dram_tensor("qT", [D, S], F32R, kind="ExternalInput")
    kT = nc.dram_tensor("kT", [D, S], F32R, kind="ExternalInput")
    vT = nc.dram_tensor("vT", [D, S], F32R, kind="ExternalInput")
    wqT = nc.dram_tensor("wqT", [D, OL], F32R, kind="ExternalInput")
    wkT = nc.dram_tensor("wkT", [D, OL], F32R, kind="ExternalInput")
    wvT = nc.dram_tensor("wvT", [D, OL], F32R, kind="ExternalInput")
    woT = nc.dram_tensor("woT", [OL, D], F32R, kind="ExternalInput")
    bqd = nc.dram_tensor("bq", [OL, 1], F32, kind="ExternalInput")
    bkd = nc.dram_tensor("bk", [OL, 1], F32, kind="ExternalInput")
    onesd = nc.dram_tensor("ones4", [128, HPC, 1], F32R, kind="ExternalInput")
    if nbias:
        mbd = nc.dram_tensor("maskb", [nbias, JB, 128], F32,
                             kind="ExternalInput")
    out = nc.dram_tensor("out", [D, S], F32, kind="ExternalOutput")

    ND = D // 128  # 8 contraction blocks

    with tile.TileContext(nc) as tc:
        with tc.tile_pool(name="consts", bufs=1) as consts:
            # resident SBUF tensors (f32r: matmul operands)
            QT = [consts.tile([128, S], F32R, name=f"QT{t}") for t in range(2)]
            KT = [consts.tile([128, S], F32R, name=f"KT{t}") for t in range(2)]
            XT = [consts.tile([128, S], F32R, name=f"XT{t}") for t in range(2)]
            Vt = [consts.tile([128, HPC * VS], F32R, name=f"V{st}")
                  for st in range(S // 128)]
            wq_t = [consts.tile([128, OL], F32R, name=f"wq{d}")
                    for d in range(ND)]
            wk_t = [consts.tile([128, OL], F32R, name=f"wk{d}")
                    for d in range(ND)]
            wv_t = [consts.tile([128, OL], F32R, name=f"wv{d}")
                    for d in range(ND)]
            wo_t = [consts.tile([128, D], F32R, name=f"wo{t}")
                    for t in range(2)]
            bq_t = [consts.tile([128, 1], F32, name=f"bq{t}") for t in range(2)]
            bk_t = [consts.tile([128, 1], F32, name=f"bk{t}") for t in range(2)]
            mb = [consts.tile([JB, 128], F32, name=f"mb{i}")
                  for i in range(nbias)]

            for t in range(2):
                nc.sync.dma_start(bq_t[t][:], bqd[t * 128:(t + 1) * 128, :])
                nc.sync.dma_start(bk_t[t][:], bkd[t * 128:(t + 1) * 128, :])

            # ---------------- phase 1: projections ----------------
            with tc.tile_pool(name="acts", bufs=10) as actp, \
                 tc.tile_pool(name="ppj", bufs=4, space="PSUM") as ppj, \
                 tc.tile_pool(name="ppv", bufs=4, space="PSUM") as ppv:

                # QT / KT: psum[o_tile 128, s 512] = sum_d wT[d,o].T @ actT[d,s]
                for dst, wt, wdram, act, bias in (
                        (QT, wq_t, wqT, qT, bq_t),
                        (KT, wk_t, wkT, kT, bk_t)):
                    for sb in range(NSB):
                        ps = [ppj.tile([128, SB], F32, tag="pj", name="ps")
                              for _ in range(2)]
                        for d in range(ND):
                            if sb == 0:
                                # interleave weight loads with the act
                                # stream so the first matmuls start early
                                nc.sync.dma_start(
                                    wt[d][:],
                                    wdram[d * 128:(d + 1) * 128, :])
                            at = actp.tile([128, SB], F32R, tag="act")
                            nc.sync.dma_start(
                                at[:],
                                act[d * 128:(d + 1) * 128,
                                    sb * SB:(sb + 1) * SB])
                            for ot in range(2):
                                nc.tensor.matmul(
                                    ps[ot][:],
                                    wt[d][:, ot * 128:(ot + 1) * 128],
                                    at[:],
                                    start=(d == 0), stop=(d == ND - 1))
                        for ot in range(2):
                            # psum -> sbuf (f32r) with per-partition bias
                            nc.vector.tensor_scalar_add(
                                dst[ot][:, sb * SB:(sb + 1) * SB],
                                ps[ot][:], bias[ot][:])

                # V: psum[s_tile 128, o 256] = vT[d, s].T @ wvT[d, o]
                for sb in range(NSB):
                    psv = [ppv.tile([128, OL], F32, tag="pv", name="psv")
                           for _ in range(4)]
                    for d in range(ND):
                        if sb == 0:
                            nc.sync.dma_start(
                                wv_t[d][:], wvT[d * 128:(d + 1) * 128, :])
                        at = actp.tile([128, SB], F32R, tag="act")
                        nc.sync.dma_start(
                            at[:],
                            vT[d * 128:(d + 1) * 128, sb * SB:(sb + 1) * SB])
                        for k in range(4):
                            nc.tensor.matmul(
                                psv[k][:],
                                at[:, k * 128:(k + 1) * 128],
                                wv_t[d][:],
                                start=(d == 0), stop=(d == ND - 1))
                    for k in range(4):
                        st = sb * 4 + k
                        v3 = Vt[st][:].rearrange("p (h c) -> p h c", c=VS)
                        nc.vector.tensor_copy(
                            v3[:, :, 0:DK],
                            psv[k][:].rearrange("p (h c) -> p h c", c=DK))
                        nc.sync.dma_start(v3[:, :, DK:VS], onesd[:])

            # mask bias tiles are first needed here; DMA them late so they
            # don't delay the projection-phase activation loads
            for i in range(nbias):
                nc.sync.dma_start(mb[i][:], mbd[i])

            # ---------------- phase 2: attention ----------------
            # All 4 heads are processed as interleaved independent streams
            # per (ib, jb) step so the PE always has ~8 queued matmuls; the
            # PV for block j is emitted after the S/exp of block j+1
            # (software pipeline), so the PE never blocks on ACT.
            with tc.tile_pool(name="pss", bufs=2, space="PSUM") as pss, \
                 tc.tile_pool(name="pso", bufs=4, space="PSUM") as pso, \
                 tc.tile_pool(name="pP", bufs=6) as pP, \
                 tc.tile_pool(name="prr", bufs=4) as prr, \
                 tc.tile_pool(name="prc", bufs=4) as prc:
                heads = [(h // 2, 64 * (h % 2)) for h in range(HPC)]
                for ib in range(NSB):
                    jl = jlists[ib]
                    Ops = [pso.tile([VS, SB], F32, tag="O", name="Ops")
                           for _ in heads]

                    def emit_pv(pend):
                        for h, jb_p, P_p, first, last in pend:
                            nc.tensor.matmul(
                                Ops[h][:],
                                Vt[jb_p][:, VS * h:VS * h + VS],
                                P_p[:],
                                start=first, stop=last)

                    # supersteps of (head-group of 2) x (2 j-blocks): one
                    # head's two S matmuls share their moving operand
                    # (QT_h[ib]) back-to-back — reused moving operands
                    # stream ~2.7x faster on the PE. PV lags one superstep
                    # per group so it never waits on ACT.
                    assert len(jl) % 2 == 0
                    pending = {0: [], 1: []}
                    for pi in range(0, len(jl), 2):
                        for g in (0, 1):
                            hs = (2 * g, 2 * g + 1)
                            made = []
                            Sps = {}
                            # the two heads of the group share one 2-bank
                            # [128,1024] score tile per j-block, so exp and
                            # the masking ops run once per j-block instead
                            # of once per head (half the ACT ops and sems)
                            for jb, sub_ops in jl[pi:pi + 2]:
                                Sps[jb] = pss.tile([JB, 2 * SB], F32,
                                                   tag="S", name="Sp")
                            # S matmuls grouped per head (rhs reuse)
                            for z, h in enumerate(hs):
                                hp, bp = heads[h]
                                for jb, sub_ops in jl[pi:pi + 2]:
                                    nc.tensor.matmul(
                                        Sps[jb][:, z * SB:(z + 1) * SB],
                                        KT[hp][bp:bp + 64,
                                               jb * JB:(jb + 1) * JB],
                                        QT[hp][bp:bp + 64,
                                               ib * SB:(ib + 1) * SB],
                                        start=True, stop=True)
                            # exp over visible/mixed runs on ACT (both
                            # heads at once via a 3D AP); mixed blocks get
                            # a 0/1 mask multiply on the (idle) GPSIMD
                            # engine, fully-masked blocks are zeroed on DVE
                            for off, (jb, sub_ops) in enumerate(
                                    jl[pi:pi + 2]):
                                Sp3 = Sps[jb][:].rearrange(
                                    "p (z i) -> p z i", z=2)
                                P = pP.tile([JB, 2 * SB], F32R, tag="P",
                                            name="P")
                                P3 = P[:].rearrange("p (z i) -> p z i", z=2)
                                for k0, k1 in _exp_runs(sub_ops):
                                    nc.scalar.activation(
                                        P3[:, :, k0 * 128:k1 * 128],
                                        Sp3[:, :, k0 * 128:k1 * 128],
                                        mybir.ActivationFunctionType.Exp)
                                for k, (stt, bidx) in enumerate(sub_ops):
                                    if stt == "x":
                                        for z in (0, 1):
                                            nc.gpsimd.tensor_mul(
                                                P[:, z * SB + k * 128:
                                                  z * SB + (k + 1) * 128],
                                                P[:, z * SB + k * 128:
                                                  z * SB + (k + 1) * 128],
                                                mb[bidx][:])
                                    elif stt == "m":
                                        nc.vector.tensor_scalar_mul(
                                            P3[:, :, k * 128:(k + 1) * 128],
                                            Sp3[:, :, k * 128:(k + 1) * 128],
                                            0.0)
                                idx = pi + off
                                for z, h in enumerate(hs):
                                    made.append(
                                        (h, jb, P[:, z * SB:(z + 1) * SB],
                                         idx == 0, idx == len(jl) - 1))
                            if pending[g]:
                                emit_pv(pending[g])
                            pending[g] = made
                    for g in (0, 1):
                        if pending[g]:
                            emit_pv(pending[g])
                    for h, (hp, bp) in enumerate(heads):
                        # r = 1/l: copy l to SBUF, then the fast custom
                        # DVE reciprocal (~18 correct bits, SBUF-only)
                        ls = prr.tile([1, SB], F32, tag="tl", name="ls")
                        nc.vector.tensor_copy(ls[:], Ops[h][DK:VS, :])
                        rr = prr.tile([1, SB], F32, tag="r", name="rr")
                        nc.vector.reciprocal_approx_fast(rr[:], ls[:])
                        Rc = prc.tile([64, SB], F32, tag="rc", name="Rc")
                        nc.gpsimd.partition_broadcast(Rc[:], rr[:])
                        nc.vector.tensor_mul(
                            XT[hp][bp:bp + 64, ib * SB:(ib + 1) * SB],
                            Ops[h][0:DK, :], Rc[:])

            # ---------------- phase 3: output projection ----------------
            for t in range(2):
                nc.sync.dma_start(wo_t[t][:], woT[t * 128:(t + 1) * 128, :])
            with tc.tile_pool(name="po", bufs=4, space="PSUM") as pout, \
                 tc.tile_pool(name="obuf", bufs=4) as outp:
                for jt in range(D // 128):
                    for sb in range(NSB):
                        ps = pout.tile([128, SB], F32, tag="po", name="pso2")
                        for ot in range(2):
                            nc.tensor.matmul(
                                ps[:],
                                wo_t[ot][:, jt * 128:(jt + 1) * 128],
                                XT[ot][:, sb * SB:(sb + 1) * SB],
                                start=(ot == 0), stop=(ot == 1))
                        ob = outp.tile([128, SB], F32, tag="ob", name="ob")
                        nc.vector.tensor_copy(ob[:], ps[:])
                        nc.sync.dma_start(
                            out[jt * 128:(jt + 1) * 128,
                                sb * SB:(sb + 1) * SB], ob[:])
    nc.finalize()
    return nc


def kernel(q, k, v, mask, wq, bq, wk, bk, wv, bv, wo, bo):
    global LAST_RUN
    q, k, v = (np.asarray(x, np.float32) for x in (q, k, v))
    wq, bq, wk, bk = (np.asarray(x, np.float32) for x in (wq, bq, wk, bk))
    wv, bv, wo, bo = (np.asarray(x, np.float32) for x in (wv, bv, wo, bo))
    mask2 = np.asarray(mask)[0, 0] != 0

    jlists, bias_tiles = _classify_mask(mask2)
    nbias = len(bias_tiles)
    maskb = (np.stack(bias_tiles).astype(np.float32)
             if nbias else None)

    scale = np.float32(1.0 / np.sqrt(DK))
    bo_eff = (bo + wo @ bv).astype(np.float32)

    # per-(batch) transposed activations, shared across the 4 group-cores
    qTs = [_round_f32r(q[b].T) for b in range(B)]
    kTs = [_round_f32r(k[b].T) for b in range(B)]
    vTs = [_round_f32r(v[b].T) for b in range(B)]

    # per-(group) weight shards
    wqTs, wkTs, wvTs, woTs, bqs, bks = [], [], [], [], [], []
    for g in range(GROUPS):
        rows = slice(g * OL, (g + 1) * OL)
        wqTs.append(_round_f32r((wq[rows] * scale).T))
        wkTs.append(_round_f32r(wk[rows].T))
        wvTs.append(_round_f32r(wv[rows].T))
        woTs.append(_round_f32r(wo[:, rows].T))
        bqs.append(np.ascontiguousarray((bq[rows] * scale)[:, None]))
        bks.append(np.ascontiguousarray(bk[rows][:, None]))

    in_maps = []
    for c in range(NCORES):
        b, g = c // GROUPS, c % GROUPS
        m = {
            "qT": qTs[b], "kT": kTs[b], "vT": vTs[b],
            "wqT": wqTs[g], "wkT": wkTs[g], "wvT": wvTs[g],
            "woT": woTs[g], "bq": bqs[g], "bk": bks[g],
            "ones4": np.ones((128, HPC, 1), np.float32),
        }
        if nbias:
            m["maskb"] = maskb
        in_maps.append(m)

    nc = _build(jlists, nbias)
    res = run_bass_kernel_spmd(nc, in_maps, core_ids=list(range(NCORES)))
    LAST_RUN = res
    if res.exec_time_ns is not None:
        print(f"HW exec time: {res.exec_time_ns} ns")

    outp = np.zeros((B, S, D), np.float32)
    for c in range(NCORES):
        b = c // GROUPS
        outp[b] += res.results[c]["out"].T
    outp += bo_eff
    return outp



# revision 10
# speedup vs baseline: 1.2146x; 1.2146x over previous
"""Multi-head attention block (B=2, S=2048, D=1024, H=16) on 8 trn2 cores.

Sharding: core c = (batch b = c//4, head-group g = c%4); each core computes
4 heads of one batch (Megatron column-shard of wq/wk/wv, row-shard of wo,
combined with data-parallel over batch). Host sums the 4 partial outputs
per batch and adds the (folded) bias.

v3: all-bf16 matmul pipeline (2.4 GHz PE, ~0.42 ns per 512-col
matmul column for bf16/f32r at 128 partitions; fp8 was tried and
rejected: the attention output is a near-uniform average of ~2k value
rows, so per-element quantization error lands ~1:1 on the final
output — e4m3's ~3%/stage blows the 2e-2 budget, bf16's 0.1% is free):
  - activations DMA'd as bf16 [dd, 128, sb, 2, 512]; weights bf16
  - Q/K/V projections: plain bf16 matmuls, contraction 128/instr
  - scores: plain bf16 matmuls [64,128]@[64,<=512], transposed S[j,i],
    diagonal blocks stream only the visible column suffix
  - exp on ACT (the bottleneck engine, ~1.09 ns/col): psum f32 -> bf16 P
    with scale=1/8 (softmax scale applied here); causal masking via a
    single shared [-30000] triangle tile added into psum scores on DVE
    before exp; fully-masked P subblocks memset to 0 on DVE
  - PV: plain bf16 per j-block: stationary V [128,65] (ones column
    gives softmax row-sums l), moving P [128,<=512]
  - normalization r=1/l on DVE + GPSIMD partition broadcast (as v1)
  - out-projection f32r (full precision), psum->bf16 out DMA
Output returned transposed [D, S] bf16 per core; host sums 4 group
partials per batch in f32 and adds bo_eff = bo + wo@bv.
"""

import numpy as np
import ml_dtypes

import concourse.bass as bass
import concourse.mybir as mybir
import concourse.tile as tile
from concourse import bacc
from concourse.bass_utils import run_bass_kernel_spmd

B, S, D, H = 2, 2048, 1024, 16
DK = D // H                  # 64
NCORES = 8
GROUPS = NCORES // B         # 4 head-groups
HPC = H // GROUPS            # 4 heads per core
OL = HPC * DK                # 256 local features
SB = 512                     # query-block (i) width
JB = 128                     # key-block (j) width
NSB = S // SB                # 4
NJB = S // JB                # 16
ND = D // 128                # 8 contraction blocks
NDD = ND // 2                # 4 DoubleRow steps
VS = DK + 1                  # V columns per head incl. ones column (65)
NEGB = -30000.0              # additive mask bias (pre exp-scale)

F32 = mybir.dt.float32
F32R = mybir.dt.float32r
BF16 = mybir.dt.bfloat16
EXP = mybir.ActivationFunctionType.Exp

BF = ml_dtypes.bfloat16

LAST_RUN = None  # stash of BassKernelResults for test harness inspection


def _round_f32r(a):
    """Round an f32 array to the f32r grid (top-20-bit float)."""
    a = np.ascontiguousarray(a, np.float32)
    u = a.view(np.uint32)
    u = (u + 0x7FF + ((u >> 12) & 1)) & np.uint32(0xFFFFF000)
    return u.view(np.float32)


def _classify_mask(mask2):
    """Derive the block schedule from the boolean mask [S, S]
    (mask2[i, j] True = visible).

    Returns (sched, bias_tiles):
      sched[ib] = list of (jb, k0, sub_ops) for j-blocks with any visible
        entry; k0 = first 128-i-subblock with any visible entry;
        sub_ops = list over the 4 i-subblocks of
        ('v', None) visible / ('m', None) fully masked / ('x', bias_idx).
      bias_tiles: [n, JB, JB] f32 additive bias (transposed [j, i]),
        0 where visible, NEGB where masked; deduplicated.
    """
    sched = []
    bias_tiles = []
    bias_keys = {}
    assert mask2.any(axis=1).all(), "mask has a fully-masked query row"
    for ib in range(NSB):
        jl = []
        for jb in range(NJB):
            sub = mask2[ib * SB:(ib + 1) * SB, jb * JB:(jb + 1) * JB]
            if not sub.any():
                continue
            sub_ops = []
            k0 = None
            for k in range(SB // JB):
                s2 = sub[k * JB:(k + 1) * JB, :]
                if s2.all():
                    sub_ops.append(("v", None))
                elif not s2.any():
                    sub_ops.append(("m", None))
                else:
                    t = np.where(s2, np.float32(0), np.float32(NEGB)).T
                    key = t.tobytes()
                    if key not in bias_keys:
                        bias_keys[key] = len(bias_tiles)
                        bias_tiles.append(t)
                    sub_ops.append(("x", bias_keys[key]))
                if k0 is None and sub_ops[-1][0] != "m":
                    k0 = k
            jl.append((jb, k0, sub_ops))
        sched.append(jl)
    return sched, bias_tiles


def _build(sched, nbias):
    nc = bacc.Bacc()

    q8d = nc.dram_tensor("q8", [NDD, 128, NSB, 2, SB], BF16,
                         kind="ExternalInput")
    k8d = nc.dram_tensor("k8", [NDD, 128, NSB, 2, SB], BF16,
                         kind="ExternalInput")
    v8d = nc.dram_tensor("v8", [NDD, 128, NSB, 2, SB], BF16,
                         kind="ExternalInput")
    wq8d = nc.dram_tensor("wq8", [128, ND, OL], BF16, kind="ExternalInput")
    wk8d = nc.dram_tensor("wk8", [128, ND, OL], BF16, kind="ExternalInput")
    wv8d = nc.dram_tensor("wv8", [128, ND, OL], BF16, kind="ExternalInput")
    wod = nc.dram_tensor("woT", [OL, D], F32R, kind="ExternalInput")
    bqd = nc.dram_tensor("bq", [128, 2], F32, kind="ExternalInput")
    bkd = nc.dram_tensor("bk", [128, 2], F32, kind="ExternalInput")
    if nbias:
        trid = nc.dram_tensor("tri", [nbias, JB, JB], F32,
                              kind="ExternalInput")
    out = nc.dram_tensor("out", [D, S], BF16, kind="ExternalOutput")

    # head h -> (tile index, partition base) in the [256, S] feature layout
    heads = [(h // 2, DK * (h % 2)) for h in range(HPC)]

    with tile.TileContext(nc) as tc:
        with tc.tile_pool(name="consts", bufs=1) as consts:
            W8q = consts.tile([128, ND, OL], BF16, name="W8q")
            W8k = consts.tile([128, ND, OL], BF16, name="W8k")
            W8v = consts.tile([128, ND, OL], BF16, name="W8v")
            WO = [consts.tile([128, D], F32R, name=f"WO{t}") for t in range(2)]
            BQ = consts.tile([128, 2], F32, name="BQ")
            BK = consts.tile([128, 2], F32, name="BK")
            TRI = [consts.tile([JB, JB], F32, name=f"TRI{i}")
                   for i in range(nbias)]
            QT = [consts.tile([128, S], BF16, name=f"QT{t}") for t in range(2)]
            KT = [consts.tile([128, S], BF16, name=f"KT{t}") for t in range(2)]
            XT = [consts.tile([128, S], F32R, name=f"XT{t}") for t in range(2)]
            # V8t[u]: j-block pair u, planes t=jb%2, per-head 128-col slot
            V8t = [consts.tile([128, 2, HPC, 128], BF16, name=f"V8_{u}")
                   for u in range(NJB // 2)]

            nc.sync.dma_start(BQ[:], bqd[:, :])
            nc.sync.dma_start(BK[:], bkd[:, :])
            for i in range(nbias):
                nc.sync.dma_start(TRI[i][:], trid[i])

            # ---------------- phase 1: projections (fp8 DoubleRow) --------
            with tc.tile_pool(name="acts", bufs=6) as actp, \
                 tc.tile_pool(name="ppj", bufs=4, space="PSUM") as ppj, \
                 tc.tile_pool(name="ppv", bufs=4, space="PSUM") as ppv:
                for sb in range(NSB):
                    # Q then K: psum[o 128, s 512] over 4 DR steps
                    for wsb, wdram, actd, dstT, bias in (
                            (W8q, wq8d, q8d, QT, BQ),
                            (W8k, wk8d, k8d, KT, BK)):
                        ps = [ppj.tile([128, SB], F32, tag="pj", name="ps")
                              for _ in range(2)]
                        for dd in range(NDD):
                            if sb == 0 and dd == 0:
                                nc.sync.dma_start(wsb[:], wdram[:, :, :])
                            at = actp.tile([128, 2, SB], BF16, tag="act",
                                           name="at")
                            nc.sync.dma_start(at[:], actd[dd, :, sb, :, :])
                            for t in range(2):
                                for ot in range(2):
                                    nc.tensor.matmul(
                                        ps[ot][:],
                                        wsb[:, 2 * dd + t,
                                            ot * 128:(ot + 1) * 128],
                                        at[:, t, :],
                                        start=(dd == 0 and t == 0),
                                        stop=(dd == NDD - 1 and t == 1))
                        for ot in range(2):
                            # out = psum + bias  (bf16)
                            nc.vector.tensor_scalar_add(
                                dstT[ot][:, sb * SB:(sb + 1) * SB],
                                ps[ot][:], bias[:, ot:ot + 1])

                    # V: psum[s 128, o 256], stationary = acts, moving = wv
                    psv = [ppv.tile([128, OL], F32, tag="pv", name="psv")
                           for _ in range(4)]
                    for dd in range(NDD):
                        if sb == 0 and dd == 0:
                            nc.sync.dma_start(W8v[:], wv8d[:, :, :])
                        at = actp.tile([128, 2, SB], BF16, tag="act",
                                       name="at")
                        nc.sync.dma_start(at[:], v8d[dd, :, sb, :, :])
                        for t in range(2):
                            for st in range(4):
                                nc.tensor.matmul(
                                    psv[st][:],
                                    at[:, t, st * 128:(st + 1) * 128],
                                    W8v[:, 2 * dd + t, :],
                                    start=(dd == 0 and t == 0),
                                    stop=(dd == NDD - 1 and t == 1))
                    for st in range(4):
                        jb = sb * 4 + st
                        u, t = jb // 2, jb % 2
                        nc.vector.tensor_copy(
                            V8t[u][:, t, :, 0:DK],
                            psv[st][:].rearrange("p (h c) -> p h c", c=DK))
                        nc.vector.memset(V8t[u][:, t, :, DK:VS], 1.0)

            for t in range(2):
                nc.sync.dma_start(WO[t][:], wod[t * 128:(t + 1) * 128, :])

            # ---------------- phase 2: attention + out-projection ---------
            with tc.tile_pool(name="pss", bufs=4, space="PSUM") as pss, \
                 tc.tile_pool(name="pso", bufs=4, space="PSUM") as pso, \
                 tc.tile_pool(name="pP", bufs=8) as pP, \
                 tc.tile_pool(name="prr", bufs=4) as prr, \
                 tc.tile_pool(name="prc", bufs=4) as prc, \
                 tc.tile_pool(name="obuf", bufs=4) as outp:

                def emit_oproj(sb):
                    for jt in range(D // 128):
                        ps = pss.tile([128, SB], F32, tag="S", name="pso2")
                        for ot in range(2):
                            nc.tensor.matmul(
                                ps[:],
                                WO[ot][:, jt * 128:(jt + 1) * 128],
                                XT[ot][:, sb * SB:(sb + 1) * SB],
                                start=(ot == 0), stop=(ot == 1))
                        ob = outp.tile([128, SB], BF16, tag="ob", name="ob")
                        nc.vector.tensor_copy(ob[:], ps[:])
                        nc.sync.dma_start(
                            out[jt * 128:(jt + 1) * 128,
                                sb * SB:(sb + 1) * SB], ob[:])

                for ib in range(NSB):
                    jl = sched[ib]
                    # group j-blocks into pairs for DoubleRow PV
                    pairs = [jl[i:i + 2] for i in range(0, len(jl), 2)]
                    Ops = [pso.tile([VS, SB], F32, tag="O", name="Ops")
                           for _ in range(HPC)]
                    npair = len(pairs)
                    pending = []  # lagged PV work: (h, u, kmin, planes)

                    def emit_pv(work, first, last):
                        for h, u, kms, Pt in work:
                            for z, (t, kmin) in enumerate(kms):
                                c0 = kmin * JB
                                nc.tensor.matmul(
                                    Ops[h][:, c0:SB],
                                    V8t[u][:, t, h, 0:VS],
                                    Pt[:, t, c0:SB],
                                    start=(first and z == 0),
                                    stop=(last and z == len(kms) - 1))

                    for pi, pair in enumerate(pairs):
                        for h in range(HPC):
                            hp, bp = heads[h]
                            Pt = pP.tile([128, 2, SB], BF16, tag="P",
                                         name="Pt")
                            for t, (jb, k0, sub_ops) in enumerate(pair):
                                Sp = pss.tile([JB, SB], F32, tag="S",
                                              name="Sp")
                                c0 = k0 * JB
                                nc.tensor.matmul(
                                    Sp[:, c0:SB],
                                    KT[hp][bp:bp + DK,
                                           jb * JB:(jb + 1) * JB],
                                    QT[hp][bp:bp + DK,
                                           ib * SB + c0:(ib + 1) * SB],
                                    start=True, stop=True)
                                for k, (stt, bidx) in enumerate(sub_ops):
                                    if stt == "x":
                                        # GPSIMD has no PSUM port; DVE adds
                                        # the mask bias into the psum scores
                                        nc.vector.tensor_add(
                                            Sp[:, k * JB:(k + 1) * JB],
                                            Sp[:, k * JB:(k + 1) * JB],
                                            TRI[bidx][:])
                                nc.scalar.activation(
                                    Pt[:, t, c0:SB], Sp[:, c0:SB], EXP,
                                    scale=0.125)
                                # zero P where masked inside the streamed
                                # suffix (PV streams cols [k0*JB, SB))
                                for k, (stt, _) in enumerate(sub_ops):
                                    if stt == "m" and k >= k0:
                                        nc.vector.memset(
                                            Pt[:, t, k * JB:(k + 1) * JB],
                                            0.0)
                            assert all(jb // 2 == pair[0][0] // 2
                                       for (jb, _, _) in pair), \
                                "jb pair not aligned to V8 plane layout"
                            # first plane of the first pair must stream the
                            # full range: its start matmul zeroes the psum.
                            # exp only wrote [k0*JB, SB) of that plane; for
                            # causal masks k0==0 there, asserted host-side.
                            kms = [(jb % 2, 0 if pi == 0 and z == 0 else k0)
                                   for z, (jb, k0, _) in enumerate(pair)]
                            if pi == 0:
                                assert pair[0][1] == 0, \
                                    "first j-block must be visible from k=0"
                            pending.append(
                                (h, pair[0][0] // 2, kms, Pt))
                        # emit previous superstep's PV after this one's S/exp
                        if pi > 0:
                            emit_pv(pending[:HPC], pi - 1 == 0,
                                    pi - 1 == npair - 1)
                            pending = pending[HPC:]
                    emit_pv(pending, npair - 1 == 0, True)
                    pending = []

                    for h in range(HPC):
                        hp, bp = heads[h]
                        ls = prr.tile([1, SB], F32, tag="tl", name="ls")
                        nc.vector.tensor_copy(ls[:], Ops[h][DK:VS, :])
                        rr = prr.tile([1, SB], F32, tag="r", name="rr")
                        nc.vector.reciprocal_approx_fast(rr[:], ls[:])
                        Rc = prc.tile([DK, SB], F32, tag="rc", name="Rc")
                        nc.gpsimd.partition_broadcast(Rc[:], rr[:])
                        nc.vector.tensor_mul(
                            XT[hp][bp:bp + DK, ib * SB:(ib + 1) * SB],
                            Ops[h][0:DK, :], Rc[:])

                    if ib > 0:
                        emit_oproj(ib - 1)
                emit_oproj(NSB - 1)
    nc.finalize()
    return nc


def kernel(q, k, v, mask, wq, bq, wk, bk, wv, bv, wo, bo):
    global LAST_RUN
    q, k, v = (np.asarray(x, np.float32) for x in (q, k, v))
    wq, bq, wk, bk = (np.asarray(x, np.float32) for x in (wq, bq, wk, bk))
    wv, bv, wo, bo = (np.asarray(x, np.float32) for x in (wv, bv, wo, bo))
    mask2 = np.asarray(mask)[0, 0] != 0

    sched, bias_tiles = _classify_mask(mask2)
    nbias = len(bias_tiles)
    trib = np.stack(bias_tiles).astype(np.float32) if nbias else None

    bo_eff = (bo + wo @ bv).astype(np.float32)

    def act_layout(x):
        # [S, D] -> bf16 [NDD, 128, NSB, 2, SB] with paired d-block planes
        a8 = np.ascontiguousarray(x.T).astype(BF)          # [D, S]
        a8 = a8.reshape(NDD, 2, 128, NSB, SB)
        return np.ascontiguousarray(a8.transpose(0, 2, 3, 1, 4))

    q8s = [act_layout(q[b]) for b in range(B)]
    k8s = [act_layout(k[b]) for b in range(B)]
    v8s = [act_layout(v[b]) for b in range(B)]

    def w_layout(w):
        # rows [OL, D] slice -> bf16 [128, ND, OL]
        w8 = np.ascontiguousarray(w.T).astype(BF)          # [D, OL]
        return np.ascontiguousarray(
            w8.reshape(ND, 128, OL).transpose(1, 0, 2))

    wq8s, wk8s, wv8s, wos, bqs, bks = [], [], [], [], [], []
    for g in range(GROUPS):
        rows = slice(g * OL, (g + 1) * OL)
        wq8s.append(w_layout(wq[rows]))
        wk8s.append(w_layout(wk[rows]))
        wv8s.append(w_layout(wv[rows]))
        wos.append(_round_f32r(wo[:, rows].T))
        bqs.append(np.ascontiguousarray(
            bq[rows].reshape(2, 128).T.astype(np.float32)))
        bks.append(np.ascontiguousarray(
            bk[rows].reshape(2, 128).T.astype(np.float32)))

    in_maps = []
    for c in range(NCORES):
        b, g = c // GROUPS, c % GROUPS
        m = {
            "q8": q8s[b], "k8": k8s[b], "v8": v8s[b],
            "wq8": wq8s[g], "wk8": wk8s[g], "wv8": wv8s[g],
            "woT": wos[g], "bq": bqs[g], "bk": bks[g],
        }
        if nbias:
            m["tri"] = trib
        in_maps.append(m)

    nc = _build(sched, nbias)
    res = run_bass_kernel_spmd(nc, in_maps, core_ids=list(range(NCORES)))
    LAST_RUN = res
    if res.exec_time_ns is not None:
        print(f"HW exec time: {res.exec_time_ns} ns")

    outp = np.zeros((B, S, D), np.float32)
    for c in range(NCORES):
        b = c // GROUPS
        outp[b] += np.asarray(res.results[c]["out"]).astype(np.float32).T
    outp += bo_eff
    return outp
